# revision 1
# baseline (speedup 1.0000x reference)
"""Trainium2 Bass kernel for nn_MHSA_37821482008969 (2D rel-pos MHSA).

Strategy: data-parallel over batch (16 batches -> 8 cores x 2). Per (batch,
head) unit, attention is computed fully transposed: S^T = K^T@Q tiles with
y (keys) on partitions, so softmax-normalization sums come from a ones-vector
matmul on PE, the attn matmul needs no transposes of exp(S), and the output
lands directly in the channel-major layout the conv output wants.

Rel-pos biases are folded into the logits accumulation as one extra K=64
matmul per tile: lhsT is a constant 0/1 selector, rhs is the skewed rel-logit
table built via a DRAM round-trip (regular strided APs implement the
rel->abs skew) plus two PE transposes for the width term.

All matmul operands are bf16 (fp32 PSUM accumulation); softmax skips the
row-max subtraction (logits are ~N(0,1), |logit| < 7, exp is safe in fp32).
"""
import numpy as np
import ml_dtypes

import concourse.bass as bass
import concourse.mybir as mybir
import concourse.tile as tile
import concourse.bacc as bacc
from concourse.bass_utils import run_bass_kernel_spmd

bf16 = ml_dtypes.bfloat16
FP32 = mybir.dt.float32
BF16 = mybir.dt.bfloat16

HEADS, D, F, DIM = 4, 128, 32, 512
L = F * F           # 1024
B_PER_CORE = 2
N_CORES = 8
AF = mybir.ActivationFunctionType

_cache = {}


def _build():
    nc = bacc.Bacc("TRN2", target_bir_lowering=False, debug=False,
                   num_devices=N_CORES)
    xin = nc.dram_tensor("xin", [B_PER_CORE, 4, 128, L], BF16, kind="ExternalInput").ap()
    wqt = nc.dram_tensor("wqt", [4, 128, DIM], BF16, kind="ExternalInput").ap()
    wkt = nc.dram_tensor("wkt", [4, 128, DIM], BF16, kind="ExternalInput").ap()
    wvt = nc.dram_tensor("wvt", [4, 128, DIM], BF16, kind="ExternalInput").ap()
    relwt = nc.dram_tensor("relwt", [128, 63], BF16, kind="ExternalInput").ap()
    relht = nc.dram_tensor("relht", [128, 63], BF16, kind="ExternalInput").ap()
    sel = nc.dram_tensor("sel", [64, 8 * 128], BF16, kind="ExternalInput").ap()
    ones_col = nc.dram_tensor("ones_col", [128, 1], BF16, kind="ExternalInput").ap()
    ones_row = nc.dram_tensor("ones_row", [1, 128], BF16, kind="ExternalInput").ap()
    ident = nc.dram_tensor("ident", [128, 128], BF16, kind="ExternalInput").ap()
    out = nc.dram_tensor("out", [B_PER_CORE, DIM, L], FP32, kind="ExternalOutput").ap()

    from contextlib import ExitStack
    ctx = ExitStack()
    with tile.TileContext(nc) as tc, ctx:
        consts = ctx.enter_context(tc.tile_pool(name="consts", bufs=1))
        xpool = ctx.enter_context(tc.tile_pool(name="xpool", bufs=2))
        vtpool = ctx.enter_context(tc.tile_pool(name="vtpool", bufs=2))
        qkpool = ctx.enter_context(tc.tile_pool(name="qkpool", bufs=2))
        rwpool = ctx.enter_context(tc.tile_pool(name="rwpool", bufs=4))
        biaspool = ctx.enter_context(tc.tile_pool(name="biaspool", bufs=2))
        ptpool = ctx.enter_context(tc.tile_pool(name="ptpool", bufs=2))
        outpool = ctx.enter_context(tc.tile_pool(name="outpool", bufs=3))
        mmps = ctx.enter_context(tc.tile_pool(name="mmps", bufs=3, space="PSUM"))
        attnps = ctx.enter_context(tc.tile_pool(name="attnps", bufs=2, space="PSUM"))
        sumsps = ctx.enter_context(tc.tile_pool(name="sumsps", bufs=1, space="PSUM"))
        tpsps = ctx.enter_context(tc.tile_pool(name="tpsps", bufs=2, space="PSUM"))
        dramw = ctx.enter_context(tc.tile_pool(name="dramw", bufs=2, space="DRAM"))
        dramh = ctx.enter_context(tc.tile_pool(name="dramh", bufs=2, space="DRAM"))

        # ---- load constants ----
        def cload(ap, shape, tag):
            t = consts.tile(shape, ap.dtype, tag=tag)
            nc.sync.dma_start(t[:], ap)
            return t
        wq_sb = [cload(wqt[c], [128, DIM], f"wq{c}") for c in range(4)]
        wk_sb = [cload(wkt[c], [128, DIM], f"wk{c}") for c in range(4)]
        wv_sb = [cload(wvt[c], [128, DIM], f"wv{c}") for c in range(4)]
        relw_sb = cload(relwt, [128, 63], "relw")
        relh_sb = cload(relht, [128, 63], "relh")
        sel_sb = cload(sel, [64, 8 * 128], "sel")
        ones_c = cload(ones_col, [128, 1], "onesc")
        ones_r = cload(ones_row, [1, 128], "onesr")
        id_sb = cload(ident, [128, 128], "ident")

        for b in range(B_PER_CORE):
            x_sb = []
            for c in range(4):
                xt = xpool.tile([128, L], BF16, tag=f"x{c}")
                nc.sync.dma_start(xt[:], xin[b, c])
                x_sb.append(xt)
            # V^T for all heads: vt_sb[yt][y(128), d(512 all heads)]
            vt_sb = []
            for yt in range(8):
                ps = mmps.tile([128, DIM], FP32, tag="mm")
                for c in range(4):
                    nc.tensor.matmul(ps[:], x_sb[c][:, yt * 128:(yt + 1) * 128],
                                     wv_sb[c][:], start=(c == 0), stop=(c == 3))
                vt = vtpool.tile([128, DIM], BF16, tag=f"vt{yt}")
                nc.vector.tensor_copy(vt[:], ps[:])
                vt_sb.append(vt)

            qs, ks, biases = [], [], []
            for h in range(HEADS):
                # ---- Q, K projections: [d(128), L] layouts ----
                q_sb = qkpool.tile([128, L], BF16, tag=f"q{h}")
                k_sb = qkpool.tile([128, L], BF16, tag=f"k{h}")
                qs.append(q_sb); ks.append(k_sb)
                for dst, w in ((q_sb, wq_sb), (k_sb, wk_sb)):
                    ps0 = mmps.tile([128, 512], FP32, tag="mm")
                    ps1 = mmps.tile([128, 512], FP32, tag="mm")
                    pss = [ps0, ps1]
                    for c in range(4):
                        for n in range(2):
                            nc.tensor.matmul(pss[n][:], w[c][:, h * 128:(h + 1) * 128],
                                             x_sb[c][:, n * 512:(n + 1) * 512],
                                             start=(c == 0), stop=(c == 3))
                    for n in range(2):
                        nc.vector.tensor_copy(dst[:, n * 512:(n + 1) * 512], pss[n][:])

            for h in range(HEADS):
                q_sb = qs[h]
                # ---- rel width logits RW[q,m], bounce via DRAM, skew ----
                skw = dramw.tile([L, 64], BF16, tag="skw")
                for j in range(8):
                    ps = mmps.tile([128, 64], FP32, tag="mm")
                    nc.tensor.matmul(ps[:, 0:63], q_sb[:, j * 128:(j + 1) * 128],
                                     relw_sb[:], start=True, stop=True)
                    rw = rwpool.tile([128, 64], BF16, tag="rw")
                    nc.vector.tensor_copy(rw[:, 0:63], ps[:, 0:63])
                    nc.sync.dma_start(skw[j * 128:(j + 1) * 128, 0:63], rw[:, 0:63])
                # ---- rel height logits RH_T[m,q] -> DRAM ----
                skh = dramh.tile([64, L], BF16, tag="skh")
                rh = rwpool.tile([64, L], BF16, tag="rh")
                for n in range(2):
                    ps = mmps.tile([64, 512], FP32, tag="mm")
                    nc.tensor.matmul(ps[0:63, :], relh_sb[:],
                                     q_sb[:, n * 512:(n + 1) * 512],
                                     start=True, stop=True)
                    nc.vector.tensor_copy(rh[0:63, n * 512:(n + 1) * 512], ps[0:63, :])
                nc.sync.dma_start(skh[0:63, :], rh[0:63, :])

                # ---- skewed reads -> bias_rhs [64, L] ----
                bias_rhs = biaspool.tile([64, L], BF16, tag=f"bias{h}")
                biases.append(bias_rhs)
                wst2 = biaspool.tile([128, 256], BF16, tag="wst2")
                src_flat = skw[:].flatten()
                dst_flat = wst2[:]
                for xh in range(4):
                    srcap = bass.AP(src_flat.tensor, src_flat.offset + 31 + xh * 2048,
                                    [[63, 32], [8192, 8], [1, 32]])
                    dstap = bass.AP(dst_flat.tensor, dst_flat.offset + xh * 32 * 256,
                                    [[256, 32], [32, 8], [1, 32]])
                    nc.sync.dma_start(dstap, srcap)
                for half in range(2):
                    tps = tpsps.tile([128, 128], BF16, tag="tps")
                    nc.tensor.transpose(tps[:], wst2[:, half * 128:(half + 1) * 128],
                                        id_sb[:])
                    wst3 = biaspool.tile([128, 128], BF16, tag=f"wst3{half}")
                    nc.vector.tensor_copy(wst3[:], tps[:])
                    for jj in range(4):
                        j = half * 4 + jj
                        nc.sync.dma_start(bias_rhs[0:32, j * 128:(j + 1) * 128],
                                          wst3[jj * 32:(jj + 1) * 32, :])
                hsrc_flat = skh[:].flatten()
                hsrc = bass.AP(hsrc_flat.tensor, hsrc_flat.offset,
                               [[1024, 32], [1056, 32], [1, 32]])
                hdst_flat = bias_rhs[:]
                hdst = bass.AP(hdst_flat.tensor, hdst_flat.offset + 32 * 1024,
                               [[1024, 32], [32, 32], [1, 32]])
                nc.sync.dma_start(hdst, hsrc)

            for h in range(HEADS):
                q_sb, k_sb, bias_rhs = qs[h], ks[h], biases[h]
                # ---- attention, per 512-wide q block ----
                for n in range(2):
                    pt_sb = []
                    for yt in range(8):
                        st = mmps.tile([128, 512], FP32, tag="mm")
                        nc.tensor.matmul(st[:], k_sb[:, yt * 128:(yt + 1) * 128],
                                         q_sb[:, n * 512:(n + 1) * 512],
                                         start=True, stop=False)
                        nc.tensor.matmul(st[:], sel_sb[:, yt * 128:(yt + 1) * 128],
                                         bias_rhs[:, n * 512:(n + 1) * 512],
                                         start=False, stop=True)
                        pt = ptpool.tile([128, 512], BF16, tag=f"pt{yt}")
                        nc.scalar.activation(pt[:], st[:], AF.Exp)
                        pt_sb.append(pt)
                    sums = sumsps.tile([1, 512], FP32, tag="sums")
                    attn = attnps.tile([128, 512], FP32, tag="attn")
                    for yt in range(8):
                        nc.tensor.matmul(sums[:], ones_c[:], pt_sb[yt][:],
                                         start=(yt == 0), stop=(yt == 7))
                    for yt in range(8):
                        nc.tensor.matmul(attn[:], vt_sb[yt][:, h * 128:(h + 1) * 128],
                                         pt_sb[yt][:], start=(yt == 0), stop=(yt == 7))
                    recip = outpool.tile([1, 512], BF16, tag="recip")
                    with nc.allow_low_precision(reason="bf16 softmax recip"):
                        nc.vector.reciprocal(recip[:], sums[:])
                    bc = mmps.tile([128, 512], FP32, tag="mm")
                    nc.tensor.matmul(bc[:], ones_r[:], recip[:], start=True, stop=True)
                    bc_sb = outpool.tile([128, 512], FP32, tag="bcsb")
                    nc.scalar.activation(bc_sb[:], bc[:], AF.Identity)
                    o_sb = outpool.tile([128, 512], FP32, tag="osb")
                    nc.vector.tensor_mul(o_sb[:], attn[:], bc_sb[:])
                    nc.sync.dma_start(
                        out[b, h * 128:(h + 1) * 128, n * 512:(n + 1) * 512], o_sb[:])

    nc.compile()
    return nc


def _prep_inputs(featuremap, w_qk, w_v, rel_height, rel_width):
    scale = D ** -0.5
    wqt = np.ascontiguousarray(w_qk[:DIM].T * scale).astype(bf16).reshape(4, 128, DIM)
    wkt = np.ascontiguousarray(w_qk[DIM:].T).astype(bf16).reshape(4, 128, DIM)
    wvt = np.ascontiguousarray(w_v.T).astype(bf16).reshape(4, 128, DIM)
    relwt = np.ascontiguousarray(rel_width.T).astype(bf16)
    relht = np.ascontiguousarray(rel_height.T[:, ::-1]).astype(bf16)
    yy = np.arange(128)
    sel = np.zeros((64, 8 * 128), np.float32)
    for yt in range(8):
        sel[yy % 32, yt * 128 + yy] = 1.0
        sel[32 + 31 - (yt * 4 + yy // 32), yt * 128 + yy] = 1.0
    sel = sel.astype(bf16)
    ones_col = np.ones((128, 1), bf16)
    ones_row = np.ones((1, 128), bf16)
    ident = np.eye(128, dtype=bf16)
    common = dict(wqt=wqt, wkt=wkt, wvt=wvt, relwt=relwt, relht=relht,
                  sel=sel, ones_col=ones_col, ones_row=ones_row, ident=ident)
    xin = featuremap.reshape(16, DIM, L).astype(bf16).reshape(
        N_CORES, B_PER_CORE, 4, 128, L)
    return [dict(common, xin=np.ascontiguousarray(xin[i])) for i in range(N_CORES)]


def kernel(featuremap, w_qk, w_v, rel_height, rel_width, _trace=False, _tmpdir=None):
    if "nc" not in _cache:
        _cache["nc"] = _build()
    nc = _cache["nc"]
    in_maps = _prep_inputs(featuremap, w_qk, w_v, rel_height, rel_width)
    res = run_bass_kernel_spmd(nc, in_maps, list(range(N_CORES)),
                               trace=_trace, tmpdir=_tmpdir)
    _cache["last_result"] = res
    full = np.concatenate([res.results[i]["out"] for i in range(N_CORES)], axis=0)
    return full.reshape(16, DIM, F, F)



# revision 5
# speedup vs baseline: 1.9108x; 1.9108x over previous
"""Trainium2 Bass kernel for nn_MHSA_37821482008969 (2D rel-pos MHSA).

Strategy: data-parallel over batch (16 batches -> 8 cores x 2). Per (batch,
head) unit, attention is computed fully transposed: S^T = K^T@Q tiles with
y (keys) on partitions, so softmax-normalization sums come from a ones-matrix
matmul on PE (replicated across all 128 partitions, so the reciprocal and
final scale run as plain full-width DVE ops), the attn matmul needs no
transposes of exp(S), and the output lands directly in the channel-major
layout the conv output wants.

Rel-pos biases are folded into the logits accumulation as one extra K=64
matmul per tile: lhsT is a constant 0/1 selector, rhs is the skewed rel-logit
table built via a DRAM round-trip (regular strided APs implement the
rel->abs skew) plus small per-j-block PE transposes for the width term.

All matmul operands are bf16 (fp32 PSUM accumulation); softmax skips the
row-max subtraction (logits are ~N(0,1), |logit| < 7, exp is safe in fp32).

The emission order software-pipelines the attention units (S-matmuls of unit
i+1 before the normalization tail of unit i) and interleaves batch 1's
projection/bias phase into batch 0's attention phase so the PE never idles
(keeps the tensor engine p-state at max clock).
"""
import numpy as np
import ml_dtypes

import concourse.bass as bass
import concourse.mybir as mybir
import concourse.tile as tile
import concourse.bacc as bacc
from concourse.bass_utils import run_bass_kernel_spmd

bf16 = ml_dtypes.bfloat16
FP32 = mybir.dt.float32
BF16 = mybir.dt.bfloat16

HEADS, D, F, DIM = 4, 128, 32, 512
L = F * F           # 1024
B_PER_CORE = 2
N_CORES = 8
AF = mybir.ActivationFunctionType

_cache = {}


def _build():
    nc = bacc.Bacc("TRN2", target_bir_lowering=False, debug=False,
                   num_devices=N_CORES)
    # host-packed layouts (see _prep_inputs)
    xin = nc.dram_tensor("xin", [B_PER_CORE, 128, 4 * L], BF16, kind="ExternalInput").ap()
    wqt = nc.dram_tensor("wqt", [128, 4 * DIM], BF16, kind="ExternalInput").ap()
    wkt = nc.dram_tensor("wkt", [128, 4 * DIM], BF16, kind="ExternalInput").ap()
    wvt = nc.dram_tensor("wvt", [128, 4 * DIM], BF16, kind="ExternalInput").ap()
    relwt = nc.dram_tensor("relwt", [128, 64], BF16, kind="ExternalInput").ap()
    relht = nc.dram_tensor("relht", [128, 64], BF16, kind="ExternalInput").ap()
    sel = nc.dram_tensor("sel", [64, 8 * 128], BF16, kind="ExternalInput").ap()
    ones = nc.dram_tensor("ones", [128, 128], BF16, kind="ExternalInput").ap()
    ident = nc.dram_tensor("ident", [128, 128], BF16, kind="ExternalInput").ap()
    out = nc.dram_tensor("out", [B_PER_CORE, DIM, L], FP32, kind="ExternalOutput").ap()

    from contextlib import ExitStack
    ctx = ExitStack()
    with tile.TileContext(nc) as tc, ctx:
        consts = ctx.enter_context(tc.tile_pool(name="consts", bufs=1))
        xpool = ctx.enter_context(tc.tile_pool(name="xpool", bufs=2))
        vtpool = ctx.enter_context(tc.tile_pool(name="vtpool", bufs=2))
        qkpool = ctx.enter_context(tc.tile_pool(name="qkpool", bufs=2))
        rwpool = ctx.enter_context(tc.tile_pool(name="rwpool", bufs=2))
        biaspool = ctx.enter_context(tc.tile_pool(name="biaspool", bufs=2))
        ptpool = ctx.enter_context(tc.tile_pool(name="ptpool", bufs=2))
        outpool = ctx.enter_context(tc.tile_pool(name="outpool", bufs=2))
        # PSUM: 8 banks total: st 3 + attn 2 + sums 1 + misc 2
        stps = ctx.enter_context(tc.tile_pool(name="stps", bufs=3, space="PSUM"))
        attnps = ctx.enter_context(tc.tile_pool(name="attnps", bufs=2, space="PSUM"))
        sumsps = ctx.enter_context(tc.tile_pool(name="sumsps", bufs=1, space="PSUM"))
        miscps = ctx.enter_context(tc.tile_pool(name="miscps", bufs=2, space="PSUM"))
        dramw = ctx.enter_context(tc.tile_pool(name="dramw", bufs=2, space="DRAM"))
        dramh = ctx.enter_context(tc.tile_pool(name="dramh", bufs=2, space="DRAM"))

        # ---- constants (DMA order matters: x0/wv first for fast start) ----
        def cload(ap, shape, tag):
            t = consts.tile(shape, ap.dtype, tag=tag)
            nc.sync.dma_start(t[:], ap)
            return t

        x_sb = [None, None]
        x_sb[0] = xpool.tile([128, 4 * L], BF16, tag="x", name="x0")
        nc.sync.dma_start(x_sb[0][:], xin[0])
        wv_sb = cload(wvt, [128, 4 * DIM], "wv")
        wq_sb = cload(wqt, [128, 4 * DIM], "wq")
        wk_sb = cload(wkt, [128, 4 * DIM], "wk")
        relw_sb = cload(relwt, [128, 64], "relw")
        relh_sb = cload(relht, [128, 64], "relh")
        sel_sb = cload(sel, [64, 8 * 128], "sel")
        ones_sb = cload(ones, [128, 128], "ones")
        id_sb = cload(ident, [128, 128], "ident")
        x_sb[1] = xpool.tile([128, 4 * L], BF16, tag="x", name="x1")
        nc.sync.dma_start(x_sb[1][:], xin[1])

        # per-batch state
        vt_sb = [[None] * 8, [None] * 8]
        qs = [[None] * HEADS, [None] * HEADS]
        ks = [[None] * HEADS, [None] * HEADS]
        biases = [[None] * HEADS, [None] * HEADS]

        def xc(b, c, lo, hi):
            # x chunk c, L-columns [lo:hi)
            return x_sb[b][:, c * L + lo: c * L + hi]

        def emit_vproj(b, yts):
            for yt in yts:
                ps = miscps.tile([128, DIM], FP32, tag="misc")
                for c in range(4):
                    nc.tensor.matmul(ps[:], xc(b, c, yt * 128, (yt + 1) * 128),
                                     wv_sb[:, c * DIM:(c + 1) * DIM],
                                     start=(c == 0), stop=(c == 3))
                vt = vtpool.tile([128, DIM], BF16, tag=f"vt{yt}")
                nc.vector.tensor_copy(vt[:], ps[:])
                vt_sb[b][yt] = vt

        def emit_qkproj(b, h):
            q_sb = qkpool.tile([128, L], BF16, tag=f"q{h}")
            k_sb = qkpool.tile([128, L], BF16, tag=f"k{h}")
            qs[b][h] = q_sb
            ks[b][h] = k_sb
            for dst, w in ((q_sb, wq_sb), (k_sb, wk_sb)):
                pss = [miscps.tile([128, 512], FP32, tag="misc", name=f"qkps{i}")
                       for i in range(2)]
                for c in range(4):
                    for n in range(2):
                        nc.tensor.matmul(pss[n][:],
                                         w[:, c * DIM + h * 128: c * DIM + (h + 1) * 128],
                                         xc(b, c, n * 512, (n + 1) * 512),
                                         start=(c == 0), stop=(c == 3))
                for n in range(2):
                    nc.vector.tensor_copy(dst[:, n * 512:(n + 1) * 512], pss[n][:])

        def emit_relbias(b, h):
            q_sb = qs[b][h]
            bias_rhs = biaspool.tile([64, L], BF16, tag=f"bias{h}")
            biases[b][h] = bias_rhs
            # ---- raw width logits rl[q, m] = q . relw, all 8 q-blocks in 1 bank
            rwps = miscps.tile([128, 512], FP32, tag="misc")
            for j in range(8):
                nc.tensor.matmul(rwps[:, j * 64:(j + 1) * 64],
                                 q_sb[:, j * 128:(j + 1) * 128], relw_sb[:],
                                 start=True, stop=True)
            rwall = rwpool.tile([128, 512], BF16, tag="rwall")
            nc.vector.tensor_copy(rwall[:], rwps[:])
            # bounce to DRAM in [q, m] row-major order (row stride 64)
            skw = dramw.tile([L, 64], BF16, tag="skw")
            skw_flat = skw[:].flatten()
            dst = bass.AP(skw_flat.tensor, skw_flat.offset,
                          [[64, 128], [8192, 8], [1, 64]])
            nc.sync.dma_start(dst, rwall[:])
            # ---- skewed reads per 128-wide x-block j, then transpose to [yw, x]
            for half in range(2):
                tps = miscps.tile([32, 512], BF16, tag="misc")
                for jj in range(4):
                    j = half * 4 + jj
                    wstj = rwpool.tile([128, 32], BF16, tag=f"wst{j}")
                    src = bass.AP(skw_flat.tensor,
                                  skw_flat.offset + 8192 * j + 31,
                                  [[2048, 4], [63, 32], [1, 32]])
                    nc.sync.dma_start(wstj[:], src)
                    nc.tensor.transpose(tps[0:32, jj * 128:(jj + 1) * 128],
                                        wstj[:], id_sb[:])
                nc.vector.tensor_copy(
                    bias_rhs[0:32, half * 512:(half + 1) * 512], tps[0:32, :])
            # ---- height logits rl_h[m, q] (relht col 63 zero-padded) ----
            rh = rwpool.tile([64, L], BF16, tag="rh")
            for n in range(2):
                ps = miscps.tile([64, 512], FP32, tag="misc")
                nc.tensor.matmul(ps[:], relh_sb[:],
                                 q_sb[:, n * 512:(n + 1) * 512],
                                 start=True, stop=True)
                nc.vector.tensor_copy(rh[:, n * 512:(n + 1) * 512], ps[:])
            skh = dramh.tile([64, L], BF16, tag="skh")
            nc.sync.dma_start(skh[:], rh[:])
            skh_flat = skh[:].flatten()
            hsrc = bass.AP(skh_flat.tensor, skh_flat.offset,
                           [[1024, 32], [1056, 32], [1, 32]])
            bias_flat = bias_rhs[:]
            hdst = bass.AP(bias_flat.tensor, bias_flat.offset + 32 * 1024,
                           [[1024, 32], [32, 32], [1, 32]])
            nc.sync.dma_start(hdst, hsrc)

        def emit_S(b, h, n):
            q_sb, k_sb, bias_rhs = qs[b][h], ks[b][h], biases[b][h]
            pts = []
            for yt in range(8):
                st = stps.tile([128, 512], FP32, tag="st")
                nc.tensor.matmul(st[:], k_sb[:, yt * 128:(yt + 1) * 128],
                                 q_sb[:, n * 512:(n + 1) * 512],
                                 start=True, stop=False)
                nc.tensor.matmul(st[:], sel_sb[:, yt * 128:(yt + 1) * 128],
                                 bias_rhs[:, n * 512:(n + 1) * 512],
                                 start=False, stop=True)
                pt = ptpool.tile([128, 512], BF16, tag=f"pt{yt}")
                nc.scalar.activation(pt[:], st[:], AF.Exp)
                pts.append(pt)
            return pts

        def emit_tail(b, h, n, pts):
            sums = sumsps.tile([128, 512], FP32, tag="sums")
            attn = attnps.tile([128, 512], FP32, tag="attn")
            for yt in range(8):
                nc.tensor.matmul(sums[:], ones_sb[:], pts[yt][:],
                                 start=(yt == 0), stop=(yt == 7))
            for yt in range(8):
                nc.tensor.matmul(attn[:], vt_sb[b][yt][:, h * 128:(h + 1) * 128],
                                 pts[yt][:], start=(yt == 0), stop=(yt == 7))
            recip = outpool.tile([128, 512], FP32, tag="recip")
            nc.vector.reciprocal_approx_fast(out=recip[:], in_=sums[:])
            o_sb = outpool.tile([128, 512], FP32, tag="osb")
            nc.vector.tensor_mul(o_sb[:], attn[:], recip[:])
            nc.sync.dma_start(
                out[b, h * 128:(h + 1) * 128, n * 512:(n + 1) * 512], o_sb[:])

        # ---- emission schedule ----
        def phase1_thunks(b):
            return ([lambda b=b: emit_vproj(b, range(0, 4)),
                     lambda b=b: emit_vproj(b, range(4, 8))] +
                    [t for h in range(HEADS) for t in
                     (lambda b=b, h=h: emit_qkproj(b, h),
                      lambda b=b, h=h: emit_relbias(b, h))])

        thunks = phase1_thunks(0) + phase1_thunks(1)
        n_consumed = 0

        def consume(upto=None, extra=0):
            nonlocal n_consumed
            target = n_consumed + extra if upto is None else max(upto, n_consumed)
            target = min(target, len(thunks))
            while n_consumed < target:
                thunks[n_consumed]()
                n_consumed += 1

        units = [(b, h, n) for b in range(B_PER_CORE)
                 for h in range(HEADS) for n in range(2)]
        prev = None
        for (b, h, n) in units:
            consume(upto=10 * b + 4 + 2 * h)   # need V + QK/REL through head h
            pts = emit_S(b, h, n)
            consume(extra=1)
            if prev is not None:
                emit_tail(*prev)
            consume(extra=1)
            prev = (b, h, n, pts)
        emit_tail(*prev)

    nc.compile()
    return nc


def _prep_inputs(featuremap, w_qk, w_v, rel_height, rel_width):
    scale = D ** -0.5
    # weights packed as [128, c_chunk*512]: w[p, c*512+d] = W.T[c*128+p, d]
    def packw(wt):  # wt: [512(c), 512(d)]
        return np.ascontiguousarray(
            wt.reshape(4, 128, DIM).transpose(1, 0, 2).reshape(128, 4 * DIM)
        ).astype(bf16)
    wqt = packw(w_qk[:DIM].T * scale)
    wkt = packw(w_qk[DIM:].T)
    wvt = packw(w_v.T)
    relwt = np.zeros((128, 64), np.float32)
    relwt[:, :63] = rel_width.T
    relwt = relwt.astype(bf16)
    relht = np.zeros((128, 64), np.float32)
    relht[:, :63] = rel_height.T[:, ::-1]
    relht = relht.astype(bf16)
    yy = np.arange(128)
    sel = np.zeros((64, 8 * 128), np.float32)
    for yt in range(8):
        sel[yy % 32, yt * 128 + yy] = 1.0
        sel[32 + 31 - (yt * 4 + yy // 32), yt * 128 + yy] = 1.0
    sel = sel.astype(bf16)
    ones = np.ones((128, 128), bf16)
    ident = np.eye(128, dtype=bf16)
    common = dict(wqt=wqt, wkt=wkt, wvt=wvt, relwt=relwt, relht=relht,
                  sel=sel, ones=ones, ident=ident)
    # x packed per batch as [128, c_chunk*L]
    xin = featuremap.reshape(16, 4, 128, L).transpose(0, 2, 1, 3).reshape(
        N_CORES, B_PER_CORE, 128, 4 * L).astype(bf16)
    return [dict(common, xin=np.ascontiguousarray(xin[i])) for i in range(N_CORES)]


def kernel(featuremap, w_qk, w_v, rel_height, rel_width, _trace=False, _tmpdir=None):
    if "nc" not in _cache:
        _cache["nc"] = _build()
    nc = _cache["nc"]
    in_maps = _prep_inputs(featuremap, w_qk, w_v, rel_height, rel_width)
    res = run_bass_kernel_spmd(nc, in_maps, list(range(N_CORES)),
                               trace=_trace, tmpdir=_tmpdir)
    _cache["last_result"] = res
    full = np.concatenate([res.results[i]["out"] for i in range(N_CORES)], axis=0)
    return full.reshape(16, DIM, F, F)
